# revision 24
# baseline (speedup 1.0000x reference)
"""Trainium2 Bass kernel for batched tanh-query attention.

Per-batch computation (B=8, one batch per NeuronCore, pure data parallel):
    q = tanh(out_state)            [Q, H]    Q=K=2048, H=128
    S = q @ history.T              [Q, K]
    P = softmax(S, axis=K)
    attn = P @ history             [Q, H]

Flash-style, no HBM intermediates, computed in the transposed orientation
S_T[k, q] so the second matmul needs no transpose of P.  Queries run in 4
quarters of 512 columns; per pair of k-blocks: S_T = ht.T @ qT (PE), exp
(ACT, the rate-limiting engine), bf16 pair/quad adds (DVE) for the softmax
denominator; MM2 (attn_T += hn.T @ expS) and the ones-matmul d rides the
next quarter in two dense batches.

Throughput notes baked into the schedule: the PE nets ~215ns per 512-col
matmul only in dense runs (p-states punish scattered singles), so input
transposes go through 4-wide PSUM bursts with a single wide copy-out, and
each quarter's epilogue is one fused job (4 dense fp32 transposes into one
PSUM tile, per-tile 1/d scaling, ONE output DMA per quarter).  The two
late tanh chunks are gated onto the exp stream via a zero bias computed
from an ex tile on the idle GpSimd engine -- otherwise the Tile scheduler
front-loads them into the ACT queue where they block the first exps while
waiting on their input DMA.  The last quarter streams its MM2 batches
inside A(3) and its epilogue scaling runs on the then-idle ACT engine to
shorten the tail.
"""

import os
import sys

os.environ.setdefault("NEURON_RT_RESET_CORES", "1")
for _p in ("/opt/trn_rl_repo", "/opt/trn_rl_repo/concourse"):
    if _p not in sys.path:
        sys.path.insert(0, _p)

import numpy as np

N_CORES = 8
SEQ = 2048
H = 128
P = 128
T = SEQ // P          # 16 seq tiles
NQ = 4                # query quarters
QW = SEQ // NQ        # 512
QTPQ = QW // P        # 4 q-tiles per quarter
NPAIR = T // 2        # 8 kb-pairs per quarter

_CACHE = {}


def _build():
    from concourse import bacc, bass, masks, mybir, tile

    f32 = mybir.dt.float32
    bf16 = mybir.dt.bfloat16
    AF = mybir.ActivationFunctionType

    nc = bacc.Bacc("TRN2", target_bir_lowering=False, debug=False,
                   num_devices=N_CORES)
    os_d = nc.dram_tensor("out_state", (SEQ, H), f32, kind="ExternalInput")
    h_d = nc.dram_tensor("history", (SEQ, H), f32, kind="ExternalInput")
    a_d = nc.dram_tensor("attn", (SEQ, H), f32, kind="ExternalOutput")

    with tile.TileContext(nc) as tc:
        with (
            tc.tile_pool(name="const", bufs=1) as constp,
            tc.tile_pool(name="big", bufs=1) as bigp,
            tc.tile_pool(name="stage", bufs=2) as stagep,
            tc.tile_pool(name="work", bufs=6) as workp,
            tc.tile_pool(name="expool", bufs=13) as expool,
            tc.tile_pool(name="dtree", bufs=8) as dtreep,
            tc.tile_pool(name="ps", bufs=2, space=bass.MemorySpace.PSUM) as psp,
            tc.tile_pool(name="psacc", bufs=2, space=bass.MemorySpace.PSUM) as pacc,
            tc.tile_pool(name="psd", bufs=2, space=bass.MemorySpace.PSUM) as psd,
        ):
            id_f32 = constp.tile([P, P], f32, tag="idf")
            masks.make_identity(nc, id_f32[:])
            id_bf = constp.tile([P, P], bf16, tag="idb")
            masks.make_identity(nc, id_bf[:])
            ones_bf = constp.tile([P, P], bf16, tag="ones")
            nc.vector.memset(ones_bf[:], 1.0)

            # persistent bf16 operands
            hn = bigp.tile([P, T, P], bf16, tag="hn")    # [k_in, t, h] natural
            ht = bigp.tile([P, T, P], bf16, tag="ht")    # [h, t, k_in]
            qT = bigp.tile([P, T, P], bf16, tag="qT")    # [h, t, q_in]

            # ---- load + preprocess (chunked so compute starts early) ----
            os_f = stagep.tile([P, T, H], f32, tag="ldin")
            hn_f = stagep.tile([P, T, H], f32, tag="ldin")
            os_v = os_d[:].rearrange("(t p) h -> p t h", p=P)
            hn_v = h_d[:].rearrange("(t p) h -> p t h", p=P)
            for j in range(4):
                sl = slice(4 * j, 4 * (j + 1))
                nc.sync.dma_start(os_f[:, sl, :], os_v[:, sl, :])
                nc.sync.dma_start(hn_f[:, sl, :], hn_v[:, sl, :])

            q_nat = stagep.tile([P, T, H], bf16, tag="qnat")
            nc.scalar.activation(q_nat[:, 0:4, :], os_f[:, 0:4, :], AF.Tanh)
            nc.vector.tensor_copy(hn[:, 0:4, :], hn_f[:, 0:4, :])
            nc.scalar.activation(q_nat[:, 4:8, :], os_f[:, 4:8, :], AF.Tanh)

            def late_prep():
                # hn casts only; the matching tanh chunks are gated onto the
                # exp stream (see gate_tanh) so they cannot block early exps
                nc.vector.tensor_copy(hn[:, 8:16, :], hn_f[:, 8:16, :])

            def gate_tanh(q, p, a):
                # zero bias computed from an ex tile on the (idle) GpSimd
                # engine: forces this tanh after exp(q,p) in the ACT order
                g = workp.tile([P, 1], f32, tag="gate", name="gate", bufs=2)
                nc.gpsimd.tensor_scalar_mul(g[:], ex_tiles[q][p][:, 0:1], 0.0)
                nc.scalar.activation(q_nat[:, a:a + 4, :], os_f[:, a:a + 4, :],
                                     AF.Tanh, bias=g[:])

            # PE-transpose a run of [128,128] bf16 tiles through one PSUM
            # tile (dense burst + single wide copy-out)
            def ptranspose_burst(dst, src, t0, n):
                tp4 = psd.tile([P, n, P], bf16, tag="dbc", name="tp4")
                for i in range(n):
                    nc.tensor.transpose(tp4[:, i, :], src[:, t0 + i, :],
                                        id_bf[:])
                nc.vector.tensor_copy(dst[:, t0:t0 + n, :], tp4[:])

            # aux work queue: input transpose bursts now, epilogues later
            aux = []

            def drain_aux(n):
                for _ in range(n):
                    if aux:
                        aux.pop(0)()

            def tp_job(kind, t0, n):
                def job():
                    src = hn if kind == "h" else q_nat
                    dst = ht if kind == "h" else qT
                    ptranspose_burst(dst, src, t0, n)
                return job

            # upfront: tiles the first A-phase pairs need.  The hn[4:8]
            # cast comes after these so its DMA wait cannot delay the DVE
            # copy-outs feeding MM1(0,0).
            ptranspose_burst(qT, q_nat, 0, QTPQ)
            ptranspose_burst(ht, hn, 0, 2)
            nc.vector.tensor_copy(hn[:, 4:8, :], hn_f[:, 4:8, :])
            aux.extend(tp_job("h", t0, n)
                       for t0, n in ((2, 4), (6, 4), (10, 4), (14, 2)))
            aux.extend(tp_job("q", t0, 4) for t0 in (4, 8, 12))

            # ---- epilogue helper: one whole quarter (dense transposes,
            # one output DMA) ----
            def emit_epi_quarter(q, aT_sb, d_sb, tail=False):
                aps4 = pacc.tile([P, QTPQ, P], f32, tag="acc", name="aps4")
                for t in range(QTPQ):
                    nc.tensor.transpose(aps4[:, t, :],
                                        aT_sb[:, P * t: P * (t + 1)],
                                        id_f32[:])
                ot4 = workp.tile([P, QTPQ, H], f32, tag="ot", name="ot4")
                for t in range(QTPQ):
                    dps = pacc.tile([P, 1], f32, tag="acc", name="dps")
                    nc.tensor.transpose(dps[:], d_sb[0:1, P * t: P * (t + 1)],
                                        id_f32[0:1, 0:1])
                    rc = workp.tile([P, 1], f32, tag="rc", name="rc")
                    nc.vector.reciprocal(rc[:], dps[:])
                    if tail:
                        # idle ACT engine does the scale in the tail
                        nc.scalar.activation(ot4[:, t, :], aps4[:, t, :],
                                             AF.Copy, bias=0.0, scale=rc[:])
                    else:
                        nc.vector.tensor_scalar_mul(ot4[:, t, :],
                                                    aps4[:, t, :], rc[:])
                out_v = a_d[q * QW: (q + 1) * QW, :].rearrange(
                    "(t p) h -> p t h", p=P)
                nc.sync.dma_start(out_v, ot4[:])

            # ---- build per-quarter phase closures ----
            ex_tiles = [[] for _ in range(NQ)]
            lvl2s = [[] for _ in range(NQ)]
            accs = [None] * NQ
            dqs = [None] * NQ
            l1prev = [None] * NQ

            def emit_pair(q, p):
                kb0 = 2 * p
                st = psp.tile([P, 2 * QW], f32, tag="st", name="st")
                rhs = qT[:, QTPQ * q: QTPQ * (q + 1), :]
                nc.tensor.matmul(st[:, 0:QW], ht[:, kb0, :], rhs,
                                 start=True, stop=True)
                nc.tensor.matmul(st[:, QW:], ht[:, kb0 + 1, :], rhs,
                                 start=True, stop=True)
                ex = expool.tile([P, 2 * QW], bf16, tag="ex", name="ex")
                nc.scalar.activation(ex[:], st[:], AF.Exp)
                ex_tiles[q].append(ex)
                # d: in-tile pair add, then quad add (DVE, bf16)
                t1 = dtreep.tile([P, QW], bf16, tag="l1", name="t1")
                nc.vector.tensor_add(t1[:], ex[:, 0:QW], ex[:, QW:])
                if l1prev[q] is None:
                    l1prev[q] = t1
                else:
                    t2 = dtreep.tile([P, QW], bf16, tag="l2", name="t2")
                    nc.vector.tensor_add(t2[:], l1prev[q][:], t1[:])
                    l1prev[q] = None
                    lvl2s[q].append(t2)

            def emit_B(q, i):
                # accumulators are allocated at first write: their lifetime
                # (B(q) start .. epilogue copy) never overlaps the next
                # quarter's, so one PSUM slot per tag suffices
                if accs[q] is None:
                    accs[q] = pacc.tile([P, QW], f32, tag="acc",
                                        name=f"acc{q}")
                    dqs[q] = psd.tile([P, QW], f32, tag="dbc", name=f"dq{q}")
                kbs = {0: range(0, 8), 1: range(8, 16), 2: range(8, 12),
                       4: range(12, 14), 5: range(14, 16)}[i]
                djs = {0: (0, 1), 1: (2, 3), 2: (2,), 4: (), 5: (3,)}[i]
                for kb in kbs:
                    nc.tensor.matmul(
                        accs[q][:], hn[:, kb, :],
                        ex_tiles[q][kb // 2][:, QW * (kb % 2): QW * (kb % 2 + 1)],
                        start=(kb == 0), stop=(kb == T - 1))
                for j in djs:
                    nc.tensor.matmul(dqs[q][:], ones_bf[:], lvl2s[q][j][:],
                                     start=(j == 0), stop=(j == 3))
                if i in (1, 5):
                    # move accumulators to SBUF, queue epilogue tiles.  For
                    # the last quarter the copy rides the (now idle) ACT
                    # engine and the epilogue runs immediately.
                    aT_sb = workp.tile([P, QW], f32, tag="atsb",
                                       name=f"aT{q}")
                    d_sb = workp.tile([P, QW], f32, tag="dsb", name=f"d{q}")
                    if i == 5:
                        nc.scalar.copy(aT_sb[:], accs[q][:])
                        nc.vector.tensor_copy(d_sb[:], dqs[q][:])
                        emit_epi_quarter(q, aT_sb, d_sb, tail=True)
                    else:
                        nc.vector.tensor_copy(aT_sb[:], accs[q][:])
                        nc.vector.tensor_copy(d_sb[:], dqs[q][:])
                        aux.append(
                            lambda a=aT_sb, d=d_sb, q=q:
                            emit_epi_quarter(q, a, d))

            # ---- emission schedule ----
            for q in range(NQ):
                for p in range(NPAIR):
                    if q == 0 and p in (1, 3, 4, 6):
                        drain_aux(1)          # ht bursts ahead of consumers
                    emit_pair(q, p)
                    if q == 0 and p == 1:
                        gate_tanh(0, 1, 8)
                    if q == 0 and p == 2:
                        late_prep()
                    if q == 0 and p == 4:
                        gate_tanh(0, 4, 12)
                    if q > 0 and p in (1, 3):
                        emit_B(q - 1, p // 2)
                    if q == NQ - 1 and p == 5:
                        emit_B(q, 0)
                    if q == NQ - 1 and p == 6:
                        emit_B(q, 2)          # kb8-11 + dMM j2
                    if q == NQ - 1 and p == 7:
                        emit_B(q, 4)          # kb12-13
                    if q == 0 and p >= 5:
                        drain_aux(1)          # qT bursts
                    if q > 0 and p in (4, 6):
                        drain_aux(1)          # epilogues, clear of B(3,0)
            emit_B(NQ - 1, 5)             # kb14-15 + dMM j3 + epilogue
            while aux:
                aux.pop(0)()

    nc.compile()
    return nc


def _get_nc():
    if "nc" not in _CACHE:
        _CACHE["nc"] = _build()
    return _CACHE["nc"]


def _run(out_state, history, trace=False):
    from concourse.bass_utils import run_bass_kernel_spmd

    nc = _get_nc()
    out_state = np.ascontiguousarray(out_state, dtype=np.float32)
    history = np.ascontiguousarray(history, dtype=np.float32)
    in_maps = [
        {"out_state": out_state[b], "history": history[b]}
        for b in range(N_CORES)
    ]
    res = run_bass_kernel_spmd(nc, in_maps, core_ids=list(range(N_CORES)),
                               trace=trace)
    attn = np.stack([res.results[b]["attn"] for b in range(N_CORES)], axis=0)
    return attn.astype(np.float32), res


def kernel(out_state, history):
    try:
        attn, _ = _run(out_state, history)
    except Exception:
        # one retry, e.g. if a previous process left a core wedged
        attn, _ = _run(out_state, history)
    return attn



# revision 55
# speedup vs baseline: 14298.3389x; 14298.3389x over previous
"""Trainium2 Bass kernel for batched tanh-query attention.

Per-batch computation (B=8, one batch per NeuronCore, pure data parallel):
    q = tanh(out_state)            [Q, H]    Q=K=2048, H=128
    S = q @ history.T              [Q, K]
    P = softmax(S, axis=K)
    attn = P @ history             [Q, H]

Flash-style, no HBM intermediates, computed in the transposed orientation
S_T[k, q] so the second matmul needs no transpose of P.  Queries run in 4
quarters of 512 columns; per pair of k-blocks: S_T = ht.T @ qT (PE), exp
(ACT, the rate-limiting engine), bf16 pair/quad adds (DVE) for the softmax
denominator; MM2 (attn_T += hn.T @ expS) and the ones-matmul d rides the
next quarter in two dense batches.

Throughput notes baked into the schedule: the PE nets ~215ns per 512-col
matmul only in dense runs (p-states punish scattered singles), so input
transposes go through 4-wide PSUM bursts with a single wide copy-out, and
each quarter's epilogue is one fused job (4 dense bf16 transposes -- half
the PE cost of fp32 -- into one PSUM tile, per-tile 1/d scaling, ONE
output DMA per quarter).  Total PE work sets the post-exp tail length, so
the epilogue stays as lean as possible.  The two
late tanh chunks are gated onto the exp stream via a zero bias computed
from an ex tile on the idle GpSimd engine -- otherwise the Tile scheduler
front-loads them into the ACT queue where they block the first exps while
waiting on their input DMA.  The last quarter streams its MM2 batches
inside A(3) and its epilogue scaling runs on the then-idle ACT engine to
shorten the tail.
"""

import os
import sys

os.environ.setdefault("NEURON_RT_RESET_CORES", "1")
for _p in ("/opt/trn_rl_repo", "/opt/trn_rl_repo/concourse"):
    if _p not in sys.path:
        sys.path.insert(0, _p)

import numpy as np

N_CORES = 8
SEQ = 2048
H = 128
P = 128
T = SEQ // P          # 16 seq tiles
NQ = 4                # query quarters
QW = SEQ // NQ        # 512
QTPQ = QW // P        # 4 q-tiles per quarter
NPAIR = T // 2        # 8 kb-pairs per quarter

_CACHE = {}


def _build():
    from concourse import bacc, bass, masks, mybir, tile

    f32 = mybir.dt.float32
    bf16 = mybir.dt.bfloat16
    AF = mybir.ActivationFunctionType

    nc = bacc.Bacc("TRN2", target_bir_lowering=False, debug=False,
                   num_devices=N_CORES)
    os_d = nc.dram_tensor("out_state", (SEQ, H), f32, kind="ExternalInput")
    h_d = nc.dram_tensor("history", (SEQ, H), f32, kind="ExternalInput")
    a_d = nc.dram_tensor("attn", (SEQ, H), f32, kind="ExternalOutput")

    with tile.TileContext(nc) as tc:
        with (
            tc.tile_pool(name="const", bufs=1) as constp,
            tc.tile_pool(name="big", bufs=1) as bigp,
            tc.tile_pool(name="stage", bufs=2) as stagep,
            tc.tile_pool(name="work", bufs=6) as workp,
            tc.tile_pool(name="expool", bufs=13) as expool,
            tc.tile_pool(name="dtree", bufs=8) as dtreep,
            tc.tile_pool(name="ps", bufs=2, space=bass.MemorySpace.PSUM) as psp,
            tc.tile_pool(name="psacc", bufs=2, space=bass.MemorySpace.PSUM) as pacc,
            tc.tile_pool(name="psd", bufs=2, space=bass.MemorySpace.PSUM) as psd,
        ):
            id_f32 = constp.tile([P, P], f32, tag="idf")
            masks.make_identity(nc, id_f32[:])
            id_bf = constp.tile([P, P], bf16, tag="idb")
            masks.make_identity(nc, id_bf[:])
            ones_bf = constp.tile([P, P], bf16, tag="ones")
            nc.vector.memset(ones_bf[:], 1.0)

            # PE p-state warm-up: ~2us of dummy matmuls inside the DMA-wait
            # window ramp the tensor engine to full clock before the real
            # transpose + MM1 chain begins (cold-start matmuls run ~3x slow)
            warm_in = workp.tile([P, 4 * P], bf16, tag="warm", name="warm",
                                 bufs=1)
            nc.vector.memset(warm_in[:], 0.0)
            for _w in range(10):
                wps = psd.tile([P, 4 * P], f32, tag="dbc", name="warm_ps")
                nc.tensor.matmul(wps[:], id_bf[:], warm_in[:],
                                 start=True, stop=True)

            # persistent bf16 operands
            hn = bigp.tile([P, T, P], bf16, tag="hn")    # [k_in, t, h] natural
            ht = bigp.tile([P, T, P], bf16, tag="ht")    # [h, t, k_in]
            qT = bigp.tile([P, T, P], bf16, tag="qT")    # [h, t, q_in]

            # ---- load + preprocess (chunked so compute starts early) ----
            os_f = stagep.tile([P, T, H], f32, tag="ldin")
            hn_f = stagep.tile([P, T, H], f32, tag="ldin")
            os_v = os_d[:].rearrange("(t p) h -> p t h", p=P)
            hn_v = h_d[:].rearrange("(t p) h -> p t h", p=P)
            for j in range(4):
                sl = slice(4 * j, 4 * (j + 1))
                nc.sync.dma_start(os_f[:, sl, :], os_v[:, sl, :])
                nc.sync.dma_start(hn_f[:, sl, :], hn_v[:, sl, :])

            q_nat = stagep.tile([P, T, H], bf16, tag="qnat")
            nc.scalar.activation(q_nat[:, 0:4, :], os_f[:, 0:4, :], AF.Tanh)
            nc.vector.tensor_copy(hn[:, 0:4, :], hn_f[:, 0:4, :])
            nc.scalar.activation(q_nat[:, 4:8, :], os_f[:, 4:8, :], AF.Tanh)

            def late_prep():
                # hn casts only, split at DMA chunk boundaries so ht(8,9)
                # transposes are not held hostage by the last inbound chunk;
                # the matching tanh chunks are gated onto the exp stream
                # (see gate_tanh) so they cannot block early exps
                nc.vector.tensor_copy(hn[:, 8:12, :], hn_f[:, 8:12, :])
                nc.vector.tensor_copy(hn[:, 12:16, :], hn_f[:, 12:16, :])

            def gate_tanh(q, p, a):
                # zero bias computed from an ex tile on the (idle) GpSimd
                # engine: forces this tanh after exp(q,p) in the ACT order
                g = workp.tile([P, 1], f32, tag="gate", name="gate", bufs=2)
                nc.gpsimd.tensor_scalar_mul(g[:], ex_tiles[q][p][:, 0:1], 0.0)
                nc.scalar.activation(q_nat[:, a:a + 4, :], os_f[:, a:a + 4, :],
                                     AF.Tanh, bias=g[:])

            # PE-transpose a run of [128,128] bf16 tiles through one PSUM
            # tile (dense burst + single wide copy-out)
            def ptranspose_burst(dst, src, t0, n):
                tp4 = psd.tile([P, n, P], bf16, tag="dbc", name="tp4")
                for i in range(n):
                    nc.tensor.transpose(tp4[:, i, :], src[:, t0 + i, :],
                                        id_bf[:])
                nc.vector.tensor_copy(dst[:, t0:t0 + n, :], tp4[:])

            # aux work queue: input transpose bursts now, epilogues later
            aux = []

            def drain_aux(n):
                for _ in range(n):
                    if aux:
                        aux.pop(0)()

            def tp_job(kind, t0, n):
                def job():
                    src = hn if kind == "h" else q_nat
                    dst = ht if kind == "h" else qT
                    ptranspose_burst(dst, src, t0, n)
                return job

            # upfront: tiles the first A-phase pairs need.  The hn[4:8]
            # cast comes after these so its DMA wait cannot delay the DVE
            # copy-outs feeding MM1(0,0).
            ptranspose_burst(qT, q_nat, 0, QTPQ)
            ptranspose_burst(ht, hn, 0, 2)
            nc.vector.tensor_copy(hn[:, 4:8, :], hn_f[:, 4:8, :])
            aux.extend(tp_job("h", t0, n)
                       for t0, n in ((2, 4), (6, 4), (10, 4), (14, 2)))
            aux.extend(tp_job("q", t0, 4) for t0 in (4, 8, 12))

            # ---- epilogue helper: one whole quarter (dense transposes,
            # one output DMA) ----
            def emit_epi_quarter(q, aT_sb, d_sb, tail=False):
                aps4 = pacc.tile([P, QTPQ, P], bf16, tag="acc",
                                 name="aps4")
                for t in range(QTPQ):
                    nc.tensor.transpose(aps4[:, t, :],
                                        aT_sb[:, P * t: P * (t + 1)],
                                        id_bf[:])
                # all four d columns into ONE PSUM tile (dense PE run,
                # no per-tile recip ping-pong), one reciprocal for all
                dps4 = pacc.tile([P, QTPQ], f32, tag="acc", name="dps4")
                for t in range(QTPQ):
                    nc.tensor.transpose(dps4[:, t:t + 1],
                                        d_sb[0:1, P * t: P * (t + 1)],
                                        id_f32[0:1, 0:1])
                rc4 = workp.tile([P, QTPQ], f32, tag="rc", name="rc4")
                nc.vector.reciprocal(rc4[:], dps4[:])
                ot4 = workp.tile([P, QTPQ, H], f32, tag="ot", name="ot4")
                # one broadcast multiply for the whole quarter: rc4 gets a
                # stride-0 trailing dim so each q-tile scales by its own 1/d
                rc4_ap = rc4[:]
                rc4_b = bass.AP(rc4_ap.tensor, rc4_ap.offset,
                                rc4_ap.ap + [[0, H]])
                nc.vector.tensor_mul(ot4[:], aps4[:], rc4_b)
                out_v = a_d[q * QW: (q + 1) * QW, :].rearrange(
                    "(t p) h -> p t h", p=P)
                nc.sync.dma_start(out_v, ot4[:])

            # ---- build per-quarter phase closures ----
            ex_tiles = [[] for _ in range(NQ)]
            lvl2s = [[] for _ in range(NQ)]
            accs = [None] * NQ
            dqs = [None] * NQ
            l1prev = [None] * NQ

            def emit_pair(q, p):
                kb0 = 2 * p
                st = psp.tile([P, 2 * QW], f32, tag="st", name="st")
                rhs = qT[:, QTPQ * q: QTPQ * (q + 1), :]
                nc.tensor.matmul(st[:, 0:QW], ht[:, kb0, :], rhs,
                                 start=True, stop=True)
                nc.tensor.matmul(st[:, QW:], ht[:, kb0 + 1, :], rhs,
                                 start=True, stop=True)
                ex = expool.tile([P, 2 * QW], bf16, tag="ex", name="ex")
                nc.scalar.activation(ex[:], st[:], AF.Exp)
                ex_tiles[q].append(ex)
                # d: in-tile pair add, then quad add (DVE, bf16)
                t1 = dtreep.tile([P, QW], bf16, tag="l1", name="t1")
                nc.vector.tensor_add(t1[:], ex[:, 0:QW], ex[:, QW:])
                if l1prev[q] is None:
                    l1prev[q] = t1
                else:
                    t2 = dtreep.tile([P, QW], bf16, tag="l2", name="t2")
                    nc.vector.tensor_add(t2[:], l1prev[q][:], t1[:])
                    l1prev[q] = None
                    lvl2s[q].append(t2)

            def emit_B(q, i):
                # accumulators are allocated at first write: their lifetime
                # (B(q) start .. epilogue copy) never overlaps the next
                # quarter's, so one PSUM slot per tag suffices
                if accs[q] is None:
                    accs[q] = pacc.tile([P, QW], f32, tag="acc",
                                        name=f"acc{q}")
                    dqs[q] = psd.tile([P, QW], f32, tag="dbc", name=f"dq{q}")
                kbs = {0: range(0, 8), 1: range(8, 16), 2: range(8, 12),
                       4: range(12, 14), 5: range(14, 16)}[i]
                djs = {0: (0, 1), 1: (2, 3), 2: (2,), 4: (), 5: (3,)}[i]
                for kb in kbs:
                    nc.tensor.matmul(
                        accs[q][:], hn[:, kb, :],
                        ex_tiles[q][kb // 2][:, QW * (kb % 2): QW * (kb % 2 + 1)],
                        start=(kb == 0), stop=(kb == T - 1))
                for j in djs:
                    nc.tensor.matmul(dqs[q][:], ones_bf[:], lvl2s[q][j][:],
                                     start=(j == 0), stop=(j == 3))
                if i in (1, 5):
                    # move accumulators to SBUF, queue epilogue tiles.  For
                    # the last quarter the copy rides the (now idle) ACT
                    # engine and the epilogue runs immediately.
                    aT_sb = workp.tile([P, QW], bf16, tag="atsb",
                                       name=f"aT{q}")
                    d_sb = workp.tile([P, QW], f32, tag="dsb", name=f"d{q}")
                    if i == 5:
                        nc.scalar.copy(aT_sb[:], accs[q][:])
                        nc.vector.tensor_copy(d_sb[:], dqs[q][:])
                        emit_epi_quarter(q, aT_sb, d_sb, tail=True)
                    else:
                        nc.vector.tensor_copy(aT_sb[:], accs[q][:])
                        nc.vector.tensor_copy(d_sb[:], dqs[q][:])
                        aux.append(
                            lambda a=aT_sb, d=d_sb, q=q:
                            emit_epi_quarter(q, a, d))

            # ---- emission schedule ----
            for q in range(NQ):
                for p in range(NPAIR):
                    if q == 0 and p in (1, 3, 4, 6):
                        drain_aux(1)          # ht bursts ahead of consumers
                    emit_pair(q, p)
                    if q == 0 and p == 1:
                        gate_tanh(0, 1, 8)
                    if q == 0 and p == 2:
                        late_prep()
                    if q == 0 and p == 4:
                        gate_tanh(0, 4, 12)
                    if q > 0 and p in (1, 3):
                        emit_B(q - 1, p // 2)
                    if q == NQ - 1 and p == 5:
                        emit_B(q, 0)
                    if q == NQ - 1 and p == 7:
                        emit_B(q, 2)          # kb8-11 + dMM j2
                    if q == 0 and p >= 5:
                        drain_aux(1)          # qT bursts
                    if q in (1, 2) and p in (4, 6):
                        drain_aux(1)          # epilogues; q3's drain in the
                                              # tail, filling exp-gated idles
            emit_B(NQ - 1, 4)             # kb12-13
            emit_B(NQ - 1, 5)             # kb14-15 + dMM j3 + epilogue
            while aux:
                aux.pop(0)()

    nc.compile()
    return nc


def _get_nc():
    if "nc" not in _CACHE:
        _CACHE["nc"] = _build()
    return _CACHE["nc"]


def _run(out_state, history, trace=False):
    from concourse.bass_utils import run_bass_kernel_spmd

    nc = _get_nc()
    out_state = np.ascontiguousarray(out_state, dtype=np.float32)
    history = np.ascontiguousarray(history, dtype=np.float32)
    in_maps = [
        {"out_state": out_state[b], "history": history[b]}
        for b in range(N_CORES)
    ]
    res = run_bass_kernel_spmd(nc, in_maps, core_ids=list(range(N_CORES)),
                               trace=trace)
    attn = np.stack([res.results[b]["attn"] for b in range(N_CORES)], axis=0)
    return attn.astype(np.float32), res


def kernel(out_state, history):
    try:
        attn, _ = _run(out_state, history)
    except Exception:
        # one retry, e.g. if a previous process left a core wedged
        attn, _ = _run(out_state, history)
    return attn

